# Initial kernel scaffold
#
"""Causal MHA + residual + LayerNorm Trainium2 kernel.

Sharding: 8 cores. Core c in 0..3 runs program A (q-chunks 0,1,6,7 of batch c);
core 4+c runs program B (q-chunks 2,3,4,5 of batch c).  Zig-zag split balances
causal work (72 kv-tile visits per head per core on both halves).  No
collectives; each core produces final LayerNorm'd rows for its q-chunks.

Inside a core (flash-style, no S x S materialization in HBM):
  - value/query transposed on-chip (PE transpose) -> [d, s] layout
  - kT/qT/v projections (fp32r matmuls, N>=256)
  - scores computed transposed  S^T[kv, q] = kT.T-free layout so that
    P = exp(S^T) is directly the PV matmul's moving operand (no transposes)
  - softmax denominator via an appended ones-column in V (M=65 PV output)
  - causal mask applied by zeroing P on diagonal tiles (gpsimd affine_select)
  - ctx^T feeds the Wo projection as lhsT directly; per-q normalization by
    1/denom is applied when copying ctx^T out of PSUM
  - residual add + LayerNorm epilogue; rstd = exp(-0.5*ln(var+eps)) keeps
    everything in the natural_log_exp activation table set (no table thrash)
"""

import sys
import threading

import numpy as np

sys.path.insert(0, "/opt/trn_rl_repo")

import concourse.bass as bass  # noqa: E402
import concourse.mybir as mybir  # noqa: E402
import concourse.tile as tile  # noqa: E402
from concourse import bacc  # noqa: E402
from concourse.masks import make_identity  # noqa: E402

F32 = mybir.dt.float32
F32R = mybir.dt.float32r
AF = mybir.ActivationFunctionType
ALU = mybir.AluOpType

B, S, D, H, K = 4, 4096, 256, 4, 64
EPS = 1e-3
SQC = 512  # q chunk size
KVT = 128  # kv tile size
MEGA = 3  # kv tiles per exp batch

Q_STARTS = {0: [0, 512, 3072, 3584], 1: [1024, 1536, 2048, 2560]}
NKV_H = {0: 32, 1: 24}  # kv tiles needed per half


def _r(ap):
    return ap.bitcast(F32R)


def build_program(q_starts, nkv, need_bo, need_gb):
    """Build one half's Bass program.  Returns compiled nc."""
    sq = len(q_starts) * SQC
    nc = bacc.Bacc("TRN2", target_bir_lowering=False, debug=False,
                   enable_asserts=False)

    d_q = nc.dram_tensor("q_shard", [sq, D], F32, kind="ExternalInput").ap()
    d_v = nc.dram_tensor("value", [nkv * KVT, D], F32, kind="ExternalInput").ap()
    d_wq = nc.dram_tensor("wq", [D, 256], F32, kind="ExternalInput").ap()
    d_wk = nc.dram_tensor("wk", [D, 256], F32, kind="ExternalInput").ap()
    d_wv = nc.dram_tensor("wv", [D, 260], F32, kind="ExternalInput").ap()
    d_wo = nc.dram_tensor("wo", [2, 128, D], F32, kind="ExternalInput").ap()
    d_bq = nc.dram_tensor("bq", [2, 128], F32, kind="ExternalInput").ap()
    d_bk = nc.dram_tensor("bk", [2, 128], F32, kind="ExternalInput").ap()
    d_bo = nc.dram_tensor("bo_eff", [1, D], F32, kind="ExternalInput").ap()
    d_g = nc.dram_tensor("gamma", [1, D], F32, kind="ExternalInput").ap()
    d_b = nc.dram_tensor("beta", [1, D], F32, kind="ExternalInput").ap()
    d_ones = nc.dram_tensor("ones4", [1, 4], F32, kind="ExternalInput").ap()
    d_out = nc.dram_tensor("out", [sq, D], F32, kind="ExternalOutput").ap()

    nqt = sq // KVT  # q tiles in shard

    from contextlib import ExitStack

    with tile.TileContext(nc) as tc, ExitStack() as stk:
        ep = stk.enter_context
        pp = ep(tc.tile_pool(name="persist", bufs=1))
        qn = ep(tc.tile_pool(name="qn", bufs=3))
        xres = ep(tc.tile_pool(name="xres", bufs=3))
        sm = ep(tc.tile_pool(name="sm", bufs=4))
        smb = ep(tc.tile_pool(name="smb", bufs=2))
        m0 = ep(tc.tile_pool(name="m0", bufs=1, space="PSUM"))
        m1 = ep(tc.tile_pool(name="m1", bufs=1, space="PSUM"))
        c0 = ep(tc.tile_pool(name="c0", bufs=1, space="PSUM"))
        c1 = ep(tc.tile_pool(name="c1", bufs=1, space="PSUM"))
        dp = ep(tc.tile_pool(name="dram", bufs=8, space="DRAM"))
        if True:
            mpool = [m0, m1]
            cpool = [c0, c1]

            # ---- constants / weights ----
            ident = pp.tile([128, 128], F32, tag="ident")
            make_identity(nc, ident)
            eps_t = pp.tile([128, 1], F32, tag="eps")
            nc.vector.memset(eps_t, EPS)
            wq_sb = pp.tile([128, 2, 256], F32R, tag="wq")
            nc.sync.dma_start(out=wq_sb, in_=d_wq.bitcast(F32R).rearrange("(c p) n -> p c n", p=128))
            wk_sb = pp.tile([128, 2, 256], F32R, tag="wk")
            nc.sync.dma_start(out=wk_sb, in_=d_wk.bitcast(F32R).rearrange("(c p) n -> p c n", p=128))
            wv_sb = pp.tile([128, 2, 260], F32R, tag="wv")
            nc.sync.dma_start(out=wv_sb, in_=d_wv.bitcast(F32R).rearrange("(c p) n -> p c n", p=128))
            wo_sb = pp.tile([128, 2, 256], F32R, tag="wo")
            nc.sync.dma_start(out=wo_sb, in_=d_wo.bitcast(F32R).rearrange("c p n -> p c n"))
            bq_sb = pp.tile([128, 2], F32, tag="bq")
            nc.sync.dma_start(out=bq_sb, in_=d_bq.rearrange("c p -> p c"))
            bk_sb = pp.tile([128, 2], F32, tag="bk")
            nc.sync.dma_start(out=bk_sb, in_=d_bk.rearrange("c p -> p c"))

            bo_b = None
            if need_bo:
                bo_b = pp.tile([128, D], F32, tag="bo_b")
                nc.gpsimd.dma_start(out=bo_b, in_=d_bo.to_broadcast((128, D)))
            g_b = b_b = None
            if need_gb:
                g_b = pp.tile([128, D], F32, tag="g_b")
                nc.gpsimd.dma_start(out=g_b, in_=d_g.to_broadcast((128, D)))
                b_b = pp.tile([128, D], F32, tag="b_b")
                nc.gpsimd.dma_start(out=b_b, in_=d_b.to_broadcast((128, D)))

            # ---- persistent regions ----
            kT = pp.tile([128, 2, nkv * KVT], F32R, tag="kT")
            qT = pp.tile([128, 2, sq], F32R, tag="qT")
            vsb = pp.tile([128, nkv, 4, 65], F32R, tag="vsb")
            ctxT = pp.tile([128, 2, sq], F32R, tag="ctxT")

            # ---- prologue (scoped pools so valT/quT space is reclaimed) ----
            with tc.tile_pool(name="prol", bufs=1) as prol, \
                 tc.tile_pool(name="xld", bufs=4) as xld:
                valT = prol.tile([128, 2, nkv * KVT], F32R, tag="valT")
                quT = prol.tile([128, 2, sq], F32R, tag="quT")

                def transpose_in(dst, src_dram, ti):
                    xt = xld.tile([128, D], F32, tag="xt")
                    nc.sync.dma_start(out=xt,
                                      in_=src_dram[ti * KVT:(ti + 1) * KVT, :])
                    tp = mpool[ti % 2].tile([128, 2, 128], F32, tag="t")
                    for dc in range(2):
                        nc.tensor.transpose(tp[:, dc, :],
                                            xt[:, dc * 128:(dc + 1) * 128], ident)
                    nc.vector.tensor_copy(dst[:, :, ti * KVT:(ti + 1) * KVT], tp)

                for j in range(nkv):
                    transpose_in(valT, d_v, j)
                for t in range(nqt):
                    transpose_in(quT, d_q, t)

                # ---- kT / qT projections: out[hk_pair, s] ----
                def proj_T(dst, srcT, w_sb, b_sb, n_s):
                    for sc in range(n_s // SQC):
                        for c in range(2):
                            ps = mpool[c].tile([128, SQC], F32, tag="t")
                            for dc in range(2):
                                nc.tensor.matmul(
                                    ps, _r(w_sb[:, dc, c * 128:(c + 1) * 128]),
                                    _r(srcT[:, dc, sc * SQC:(sc + 1) * SQC]),
                                    start=(dc == 0), stop=(dc == 1))
                            nc.vector.tensor_scalar(
                                out=dst[:, c, sc * SQC:(sc + 1) * SQC], in0=ps,
                                scalar1=b_sb[:, c:c + 1], scalar2=None,
                                op0=ALU.add)

                proj_T(kT, valT, wk_sb, bk_sb, nkv * KVT)
                proj_T(qT, quT, wq_sb, bq_sb, sq)

                # ---- v projection: natural [s, (h,65)] with ones column ----
                for j in range(nkv):
                    ps = cpool[j % 2].tile([128, 260], F32, tag="t")
                    for dc in range(2):
                        nc.tensor.matmul(
                            ps, _r(valT[:, dc, j * KVT:(j + 1) * KVT]),
                            _r(wv_sb[:, dc, :]), start=(dc == 0), stop=(dc == 1))
                    nc.vector.tensor_copy(
                        vsb[:, j, :, :].rearrange("p a b -> p (a b)"), ps)
                    nc.gpsimd.dma_start(
                        out=vsb[:, j, :, 64:65].rearrange("p a b -> p (a b)"),
                        in_=d_ones.bitcast(F32R).to_broadcast((128, 4)))

            # ---- main-loop P pools (opened after prologue space freed) ----
            pp0 = stk.enter_context(tc.tile_pool(name="p0", bufs=3))
            pp1 = stk.enter_context(tc.tile_pool(name="p1", bufs=3))
            ppool = [pp0, pp1]

            # ---- main attention loop ----
            for qci, q0 in enumerate(q_starts):
                ext = (q0 + SQC) // KVT
                for pair in range(2):
                    ctx_ps = [cpool[par].tile([65, SQC], F32, tag="t",
                                              name=f"ctx_{qci}_{pair}_{par}")
                              for par in range(2)]
                    batches = [list(range(s0, min(s0 + MEGA, ext)))
                               for s0 in range(0, ext, MEGA)]
                    for bi, batch in enumerate(batches):
                        nb = len(batch)
                        for par in range(2):
                            mega = mpool[par].tile([128, nb, SQC], F32, tag="t")
                            for jj, j in enumerate(batch):
                                nc.tensor.matmul(
                                    mega[:, jj, :],
                                    _r(kT[64 * par:64 * par + 64, pair,
                                          j * KVT:(j + 1) * KVT]),
                                    _r(qT[64 * par:64 * par + 64, pair,
                                          qci * SQC:(qci + 1) * SQC]),
                                    start=True, stop=True)
                            P = ppool[par].tile([128, MEGA, SQC], F32R, tag="P")
                            nc.scalar.activation(P[:, :nb, :], mega, AF.Exp,
                                                 scale=0.125)
                            for jj, j in enumerate(batch):
                                if (j + 1) * KVT > q0:  # diagonal tile
                                    nc.gpsimd.affine_select(
                                        out=P[:, jj, :], in_=P[:, jj, :],
                                        compare_op=ALU.is_ge, fill=0.0,
                                        base=q0 - j * KVT,
                                        channel_multiplier=-1,
                                        pattern=[[1, SQC]])
                            for jj, j in enumerate(batch):
                                nc.tensor.matmul(
                                    ctx_ps[par],
                                    _r(vsb[:, j, pair * 2 + par, :]),
                                    _r(P[:, jj, :]),
                                    start=(bi == 0 and jj == 0),
                                    stop=(bi == len(batches) - 1 and jj == nb - 1))
                    # normalize by 1/denom and store ctx^T
                    for par in range(2):
                        dsb = smb.tile([1, SQC], F32, tag="dsb")
                        nc.vector.tensor_copy(dsb, ctx_ps[par][64:65, :])
                        dr1 = dp.tile([1, SQC], F32, tag="dr")
                        nc.sync.dma_start(out=dr1, in_=dsb)
                        dsc = sm.tile([128, 4], F32, tag="dsc")
                        nc.sync.dma_start(
                            out=dsc, in_=dr1[0, :].rearrange("(c p) -> p c", p=128))
                        rec = sm.tile([128, 4], F32, tag="rec")
                        nc.vector.reciprocal(rec, dsc)
                        dr2 = dp.tile([1, SQC], F32, tag="dr")
                        nc.sync.dma_start(
                            out=dr2[0, :].rearrange("(c p) -> p c", p=128), in_=rec)
                        rb = smb.tile([64, SQC], F32, tag="rb")
                        nc.gpsimd.dma_start(out=rb, in_=dr2.to_broadcast((64, SQC)))
                        nc.vector.tensor_mul(
                            ctxT[64 * par:64 * par + 64, pair,
                                 qci * SQC:(qci + 1) * SQC],
                            ctx_ps[par][0:64, :], rb)

                # ---- output projection + residual + LayerNorm for this chunk ----
                for st in range(SQC // 128):
                    gt = qci * (SQC // 128) + st  # shard s-tile index
                    mh = cpool[st % 2].tile([128, D], F32, tag="t")
                    for c in range(2):
                        nc.tensor.matmul(
                            mh, _r(ctxT[:, c, gt * 128:(gt + 1) * 128]),
                            _r(wo_sb[:, c, :]), start=(c == 0), stop=(c == 1))
                    qnat = qn.tile([128, D], F32, tag="qn")
                    nc.sync.dma_start(out=qnat,
                                      in_=d_q[gt * 128:(gt + 1) * 128, :])
                    x = xres.tile([128, D], F32, tag="x")
                    nc.vector.tensor_add(x, mh, qnat)
                    if need_bo:
                        nc.vector.tensor_add(x, x, bo_b)
                    stats = sm.tile([128, 6], F32, tag="st")
                    nc.vector.bn_stats(out=stats, in_=x)
                    mv = sm.tile([128, 2], F32, tag="mv")
                    nc.vector.bn_aggr(out=mv, in_=stats)
                    lnv = sm.tile([128, 1], F32, tag="lnv")
                    nc.scalar.activation(lnv, mv[:, 1:2], AF.Ln, bias=eps_t,
                                         scale=1.0)
                    rstd = sm.tile([128, 1], F32, tag="rstd")
                    nc.scalar.activation(rstd, lnv, AF.Exp, scale=-0.5)
                    nc.vector.tensor_scalar(
                        out=x, in0=x, scalar1=mv[:, 0:1], scalar2=rstd,
                        op0=ALU.subtract, op1=ALU.mult)
                    if need_gb:
                        nc.vector.tensor_mul(x, x, g_b)
                        nc.vector.tensor_add(x, x, b_b)
                    nc.sync.dma_start(out=d_out[gt * 128:(gt + 1) * 128, :], in_=x)

    nc.compile()
    return nc


# ---------------------------------------------------------------------------
# host side
# ---------------------------------------------------------------------------

_CACHE = {}


def _prep_weights(Wq, bq, Wk, bk, Wv, bv, Wo, bo, gamma, beta):
    wq = np.ascontiguousarray(Wq.reshape(D, 256), np.float32)
    wk = np.ascontiguousarray(Wk.reshape(D, 256), np.float32)
    wv = np.zeros((D, 260), np.float32)
    wv.reshape(D, 4, 65)[:, :, :64] = Wv.reshape(D, 4, 64)
    wo = np.ascontiguousarray(Wo.reshape(256, D).reshape(2, 128, D), np.float32)
    bq2 = np.ascontiguousarray(bq.reshape(2, 128), np.float32)
    bk2 = np.ascontiguousarray(bk.reshape(2, 128), np.float32)
    # bv folds past attention:  attn_norm @ (v + bv) = attn_norm @ v + bv
    # then (ctx + bv) @ Wo + bo = ctx@Wo + (bv@Wo + bo)
    bo_eff = (bv.reshape(256) @ Wo.reshape(256, D) + bo).astype(np.float32)
    need_bo = bool(np.any(bo_eff))
    need_gb = bool(np.any(gamma != 1.0) or np.any(beta != 0.0))
    return dict(wq=wq, wk=wk, wv=wv, wo=wo, bq=bq2, bk=bk2,
                ones4=np.ones((1, 4), np.float32),
                bo_eff=bo_eff.reshape(1, D),
                gamma=np.ascontiguousarray(gamma.reshape(1, D), np.float32),
                beta=np.ascontiguousarray(beta.reshape(1, D), np.float32),
                need_bo=need_bo, need_gb=need_gb)


def _get_programs(need_bo, need_gb):
    key = (need_bo, need_gb)
    if key not in _CACHE:
        ncs = {}
        for half in (0, 1):
            ncs[half] = build_program(Q_STARTS[half], NKV_H[half],
                                      need_bo, need_gb)
        _CACHE[key] = ncs
    return _CACHE[key]


def _make_runner(nc, devices):
    """Build a jit'd shard_map runner for `nc` over a specific device list.

    Mirrors concourse.bass2jax.run_bass_via_pjrt but with an explicit device
    subset so two programs can run concurrently on disjoint cores.
    """
    import jax
    from jax.experimental.shard_map import shard_map
    from jax.sharding import Mesh, PartitionSpec
    from concourse.bass2jax import (_bass_exec_p, install_neuronx_cc_hook,
                                    partition_id_tensor)

    install_neuronx_cc_hook()
    n_cores = len(devices)

    partition_name = (nc.partition_id_tensor.name
                      if nc.partition_id_tensor else None)
    in_names, out_names, out_avals, zero_outs = [], [], [], []
    for alloc in nc.m.functions[0].allocations:
        if not isinstance(alloc, mybir.MemoryLocationSet):
            continue
        name = alloc.memorylocations[0].name
        if alloc.kind == "ExternalInput":
            if name != partition_name:
                in_names.append(name)
        elif alloc.kind == "ExternalOutput":
            out_names.append(name)
            shape = tuple(alloc.tensor_shape)
            dtype = mybir.dt.np(alloc.dtype)
            out_avals.append(jax.core.ShapedArray(shape, dtype))
            zero_outs.append(np.zeros(shape, dtype))
    n_params = len(in_names)
    all_names = in_names + out_names
    if partition_name is not None:
        all_names = all_names + [partition_name]

    def _body(*args):
        operands = list(args)
        if partition_name is not None:
            operands.append(partition_id_tensor())
        outs = _bass_exec_p.bind(
            *operands, out_avals=tuple(out_avals), in_names=tuple(all_names),
            out_names=tuple(out_names), lowering_input_output_aliases=(),
            sim_require_finite=True, sim_require_nnan=True, nc=nc)
        return tuple(outs)

    donate = tuple(range(n_params, n_params + len(out_names)))
    mesh = Mesh(np.asarray(devices), ("core",))
    in_specs = (PartitionSpec("core"),) * (n_params + len(out_names))
    out_specs = (PartitionSpec("core"),) * len(out_names)
    fn = jax.jit(shard_map(_body, mesh=mesh, in_specs=in_specs,
                           out_specs=out_specs, check_rep=False),
                 donate_argnums=donate, keep_unused=True)

    def run(in_maps):
        assert len(in_maps) == n_cores
        concat_in = [np.concatenate([np.asarray(m[n]) for m in in_maps], axis=0)
                     for n in in_names]
        concat_zero = [np.zeros((n_cores * z.shape[0], *z.shape[1:]), z.dtype)
                       for z in zero_outs]
        out_arrs = fn(*concat_in, *concat_zero)
        return out_arrs, out_names, out_avals

    run.fn = fn
    run.mesh = mesh
    run.in_names = in_names
    run.zero_outs = zero_outs
    run.n_cores = n_cores
    return run


_RUNNERS = {}
_LAST_IN_MAPS = {}


def kernel(query, value, Wq, bq, Wk, bk, Wv, bv, Wo, bo, gamma, beta):
    import jax
    query = np.asarray(query, np.float32)
    value = np.asarray(value, np.float32)
    w = _prep_weights(np.asarray(Wq), np.asarray(bq), np.asarray(Wk),
                      np.asarray(bk), np.asarray(Wv), np.asarray(bv),
                      np.asarray(Wo), np.asarray(bo), np.asarray(gamma),
                      np.asarray(beta))
    ncs = _get_programs(w["need_bo"], w["need_gb"])

    rkey = (w["need_bo"], w["need_gb"])
    if rkey not in _RUNNERS:
        devs = jax.devices()
        _RUNNERS[rkey] = {
            0: _make_runner(ncs[0], devs[0:4]),
            1: _make_runner(ncs[1], devs[4:8]),
        }
    runners = _RUNNERS[rkey]

    wmap = {k: w[k] for k in ("wq", "wk", "wv", "wo", "bq", "bk", "bo_eff",
                              "gamma", "beta", "ones4")}
    in_maps = {0: [], 1: []}
    for half in (0, 1):
        for b in range(B):
            qs = np.concatenate([query[b, q0:q0 + SQC]
                                 for q0 in Q_STARTS[half]], axis=0)
            m = dict(wmap)
            m["q_shard"] = np.ascontiguousarray(qs)
            m["value"] = np.ascontiguousarray(value[b, :NKV_H[half] * KVT])
            in_maps[half].append(m)
        _LAST_IN_MAPS[half] = in_maps[half]

    results = {}

    def _dispatch(half):
        results[half] = runners[half].__call__(in_maps[half])

    # dispatch A then B; jax dispatch is async so both halves overlap on
    # their disjoint device sets. Use threads to overlap even blocking parts.
    th = threading.Thread(target=_dispatch, args=(0,))
    th.start()
    _dispatch(1)
    th.join()

    out = np.empty((B, S, D), np.float32)
    for half in (0, 1):
        arrs, out_names, out_avals = results[half]
        oi = out_names.index("out")
        full = np.asarray(arrs[oi]).reshape(4, len(Q_STARTS[half]) * SQC, D)
        for b in range(B):
            for ci, q0 in enumerate(Q_STARTS[half]):
                out[b, q0:q0 + SQC] = full[b, ci * SQC:(ci + 1) * SQC]
    return out



# revision 28
# speedup vs baseline: 242.6442x; 242.6442x over previous
"""Causal MHA + residual + LayerNorm Trainium2 kernel.

Sharding: 8 cores. Core c in 0..3 runs program A (q-chunks 0,1,6,7 of batch c);
core 4+c runs program B (q-chunks 2,3,4,5 of batch c).  Zig-zag split balances
causal work (72 kv-tile visits per head per core on both halves).  No
collectives; each core produces final LayerNorm'd rows for its q-chunks.

Inside a core (flash-style, no S x S materialization in HBM):
  - value/query transposed on-chip (PE transpose) -> [d, s] layout
  - kT/qT/v projections (fp32r matmuls, N>=256)
  - scores computed transposed  S^T[kv, q] = kT.T-free layout so that
    P = exp(S^T) is directly the PV matmul's moving operand (no transposes)
  - softmax denominator via an appended ones-column in V (M=65 PV output)
  - causal mask applied by zeroing P on diagonal tiles (gpsimd affine_select)
  - ctx^T feeds the Wo projection as lhsT directly; per-q normalization by
    1/denom is applied when copying ctx^T out of PSUM
  - residual add + LayerNorm epilogue; rstd = exp(-0.5*ln(var+eps)) keeps
    everything in the natural_log_exp activation table set (no table thrash)
"""

import sys
import threading

import numpy as np

sys.path.insert(0, "/opt/trn_rl_repo")

import concourse.bass as bass  # noqa: E402
import concourse.mybir as mybir  # noqa: E402
import concourse.tile as tile  # noqa: E402
from concourse import bacc  # noqa: E402
from concourse.masks import make_identity  # noqa: E402

F32 = mybir.dt.float32
F32R = mybir.dt.float32r
AF = mybir.ActivationFunctionType
ALU = mybir.AluOpType

B, S, D, H, K = 4, 4096, 256, 4, 64
EPS = 1e-3
SQC = 512  # q chunk size
KVT = 128  # kv tile size
MEGA = 2  # kv tiles per exp batch

Q_STARTS = {0: [0, 1536, 2048, 3584], 1: [512, 1024, 2560, 3072]}
NKV_H = {0: 32, 1: 28}  # kv tiles needed per half


def _r(ap):
    return ap.bitcast(F32R)


def build_program(q_starts, nkv, need_bo, need_gb, reps=1):
    """Build one half's Bass program.  Returns compiled nc.

    reps>1 repeats the whole computation back-to-back inside one NEFF;
    used by test.py for slope-based HW timing (amortizes dispatch).
    """
    sq = len(q_starts) * SQC
    nc = bacc.Bacc("TRN2", target_bir_lowering=False, debug=False,
                   enable_asserts=False)

    d_q = nc.dram_tensor("q_shard", [sq, D], F32, kind="ExternalInput").ap()
    d_v = nc.dram_tensor("value", [nkv * KVT, D], F32, kind="ExternalInput").ap()
    d_wq = nc.dram_tensor("wq", [D, 256], F32, kind="ExternalInput").ap()
    d_wk = nc.dram_tensor("wk", [D, 256], F32, kind="ExternalInput").ap()
    d_wv = nc.dram_tensor("wv", [D, 260], F32, kind="ExternalInput").ap()
    d_wo = nc.dram_tensor("wo", [2, 128, D], F32, kind="ExternalInput").ap()
    d_bq = nc.dram_tensor("bq", [2, 128], F32, kind="ExternalInput").ap()
    d_bk = nc.dram_tensor("bk", [2, 128], F32, kind="ExternalInput").ap()
    d_bo = nc.dram_tensor("bo_eff", [1, D], F32, kind="ExternalInput").ap()
    d_g = nc.dram_tensor("gamma", [1, D], F32, kind="ExternalInput").ap()
    d_b = nc.dram_tensor("beta", [1, D], F32, kind="ExternalInput").ap()
    d_ones = nc.dram_tensor("ones4", [1, 4], F32, kind="ExternalInput").ap()
    d_out = nc.dram_tensor("out", [sq, D], F32, kind="ExternalOutput").ap()

    nqt = sq // KVT  # q tiles in shard

    from contextlib import ExitStack

    with tile.TileContext(nc) as tc:
        for _rep in range(reps):
            _build_body(nc, tc, ExitStack, _rep, q_starts, nkv, sq, nqt,
                        need_bo, need_gb, d_q, d_v, d_wq, d_wk, d_wv, d_wo,
                        d_bq, d_bk, d_bo, d_g, d_b, d_ones, d_out)

    nc.compile()
    return nc


def _build_body(nc, tc, ExitStack, _rep, q_starts, nkv, sq, nqt, need_bo,
                need_gb, d_q, d_v, d_wq, d_wk, d_wv, d_wo, d_bq, d_bk, d_bo,
                d_g, d_b, d_ones, d_out):
    with ExitStack() as stk:
        ep = stk.enter_context
        sfx = f"_{_rep}" if _rep else ""
        pp = ep(tc.tile_pool(name="persist" + sfx, bufs=1))
        qn = ep(tc.tile_pool(name="qn" + sfx, bufs=3))
        xres = ep(tc.tile_pool(name="xres" + sfx, bufs=16))
        sm = ep(tc.tile_pool(name="sm" + sfx, bufs=4))
        smb = ep(tc.tile_pool(name="smb" + sfx, bufs=3))
        m0 = ep(tc.tile_pool(name="m0" + sfx, bufs=1, space="PSUM"))
        m1 = ep(tc.tile_pool(name="m1" + sfx, bufs=1, space="PSUM"))
        c0 = ep(tc.tile_pool(name="c0" + sfx, bufs=1, space="PSUM"))
        c1 = ep(tc.tile_pool(name="c1" + sfx, bufs=1, space="PSUM"))
        pr = ep(tc.tile_pool(name="pr" + sfx, bufs=2, space="PSUM"))
        vtp = ep(tc.tile_pool(name="vtp" + sfx, bufs=2))
        qtp = ep(tc.tile_pool(name="qtp" + sfx, bufs=2))
        xld = ep(tc.tile_pool(name="xld" + sfx, bufs=4))
        pp0 = ep(tc.tile_pool(name="p0" + sfx, bufs=3))
        pp1 = ep(tc.tile_pool(name="p1" + sfx, bufs=3))
        if True:
            mpool = [m0, m1]
            cpool = [c0, c1]
            ppool = [pp0, pp1]

            # pin the act table to the set containing BOTH exp and ln so
            # the compiler's table-load pass never needs to switch mid-run
            nc.scalar.add_instruction(mybir.InstLoadActFuncSet(
                name=f"I-{nc.next_id()}", engine=mybir.EngineType.Activation,
                act_func_set_id=6, ins=[], outs=[]))

            # ---- constants / weights ----
            ident = pp.tile([128, 128], F32, tag="ident")
            make_identity(nc, ident)
            eps_t = pp.tile([128, 1], F32, tag="eps")
            nc.vector.memset(eps_t, EPS)
            wq_sb = pp.tile([128, 2, 256], F32R, tag="wq")
            wk_sb = pp.tile([128, 2, 256], F32R, tag="wk")
            wv_sb = pp.tile([128, 2, 260], F32R, tag="wv")
            wo_sb = pp.tile([128, 2, 256], F32R, tag="wo")
            bq_sb = pp.tile([128, 2], F32, tag="bq")
            bk_sb = pp.tile([128, 2], F32, tag="bk")

            bo_b = None
            if need_bo:
                bo_b = pp.tile([128, D], F32, tag="bo_b")
            g_b = b_b = None
            if need_gb:
                g_b = pp.tile([128, D], F32, tag="g_b")
                b_b = pp.tile([128, D], F32, tag="b_b")

            def late_weight_loads():
                nc.sync.dma_start(out=wq_sb, in_=d_wq.bitcast(F32R).rearrange("(c p) n -> p c n", p=128))
                nc.sync.dma_start(out=wk_sb, in_=d_wk.bitcast(F32R).rearrange("(c p) n -> p c n", p=128))
                nc.sync.dma_start(out=bq_sb, in_=d_bq.rearrange("c p -> p c"))
                nc.sync.dma_start(out=bk_sb, in_=d_bk.rearrange("c p -> p c"))
                nc.sync.dma_start(out=wv_sb, in_=d_wv.bitcast(F32R).rearrange("(c p) n -> p c n", p=128))
                nc.sync.dma_start(out=wo_sb, in_=d_wo.bitcast(F32R).rearrange("c p n -> p c n"))
                if need_bo:
                    nc.gpsimd.dma_start(out=bo_b, in_=d_bo.to_broadcast((128, D)))
                if need_gb:
                    nc.gpsimd.dma_start(out=g_b, in_=d_g.to_broadcast((128, D)))
                    nc.gpsimd.dma_start(out=b_b, in_=d_b.to_broadcast((128, D)))

            # ---- persistent regions ----
            kT = pp.tile([128, 2, nkv * KVT], F32R, tag="kT")
            qT = pp.tile([128, 2, sq], F32R, tag="qT")
            vsb = pp.tile([128, nkv, 4, 65], F32R, tag="vsb")
            ctxT = pp.tile([128, 2, sq], F32R, tag="ctxT")

            # ---- streaming prologue tasks (interleaved with attention) ----
            def transpose_into(dst_slice, src_dram_slice, pi):
                xt = xld.tile([128, D], F32, tag="xt")
                nc.sync.dma_start(out=xt, in_=src_dram_slice)
                tp = pr.tile([128, 2, 128], F32, tag="t")
                for dc in range(2):
                    nc.tensor.transpose(tp[:, dc, :],
                                        xt[:, dc * 128:(dc + 1) * 128], ident)
                nc.vector.tensor_copy(dst_slice, tp)

            # prep subtasks (~0.9us PE each, interleaved between batches):
            #   ("kvt", g, h): transpose kv tiles 4g+2h, 4g+2h+1 into vt[g]
            #   ("kvp", g):    kT projection for group g
            #   ("kvv", g):    vsb projection for group g's 4 tiles
            #   ("qt", qci, h) / ("qp", qci): same for the q side
            vts, qts = {}, {}

            def sub_kvt(g, h):
                if h == 0:
                    vts[g] = vtp.tile([128, 2, 512], F32R, tag="vt", name=f"vt_{g}")
                vt = vts[g]
                for t in (2 * h, 2 * h + 1):
                    ti = g * 4 + t
                    transpose_into(vt[:, :, t * 128:(t + 1) * 128],
                                   d_v[ti * KVT:(ti + 1) * KVT, :], t)

            def sub_kvp(g):
                vt = vts[g]
                for c in range(2):
                    ps = pr.tile([128, 512], F32, tag="t")
                    for dc in range(2):
                        nc.tensor.matmul(ps, _r(wk_sb[:, dc, c * 128:(c + 1) * 128]),
                                         _r(vt[:, dc, :]),
                                         start=(dc == 0), stop=(dc == 1))
                    nc.vector.tensor_scalar(
                        out=kT[:, c, g * 512:(g + 1) * 512], in0=ps,
                        scalar1=bk_sb[:, c:c + 1], scalar2=None, op0=ALU.add)

            def sub_kvv(g):
                vt = vts.pop(g)
                for t in range(4):
                    ti = g * 4 + t
                    ps = pr.tile([128, 260], F32, tag="t")
                    for dc in range(2):
                        nc.tensor.matmul(ps, _r(vt[:, dc, t * 128:(t + 1) * 128]),
                                         _r(wv_sb[:, dc, :]),
                                         start=(dc == 0), stop=(dc == 1))
                    nc.vector.tensor_copy(
                        vsb[:, ti, :, :].rearrange("p a b -> p (a b)"), ps)
                    nc.gpsimd.dma_start(
                        out=vsb[:, ti, :, 64:65].rearrange("p a b -> p (a b)"),
                        in_=d_ones.bitcast(F32R).to_broadcast((128, 4)))

            def sub_qt(qci, h):
                if h == 0:
                    qts[qci] = qtp.tile([128, 2, 512], F32R, tag="qt", name=f"qt_{qci}")
                qt = qts[qci]
                for t in (2 * h, 2 * h + 1):
                    gt = qci * 4 + t
                    transpose_into(qt[:, :, t * 128:(t + 1) * 128],
                                   d_q[gt * KVT:(gt + 1) * KVT, :], t)

            def sub_qp(qci):
                qt = qts.pop(qci)
                for c in range(2):
                    ps = pr.tile([128, 512], F32, tag="t")
                    for dc in range(2):
                        nc.tensor.matmul(ps, _r(wq_sb[:, dc, c * 128:(c + 1) * 128]),
                                         _r(qt[:, dc, :]),
                                         start=(dc == 0), stop=(dc == 1))
                    nc.vector.tensor_scalar(
                        out=qT[:, c, qci * 512:(qci + 1) * 512], in0=ps,
                        scalar1=bq_sb[:, c:c + 1], scalar2=None, op0=ALU.add)

            SUBS = {"kvt": sub_kvt, "kvp": sub_kvp, "kvv": sub_kvv,
                    "qt": sub_qt, "qp": sub_qp}

            # just-in-time prep schedule: emit each prep task LOOKAHEAD
            # batch-slots before the first batch that consumes its output.
            exts = [(q0 + SQC) // KVT for q0 in q_starts]
            stream = []  # (qci, pair, bi)
            for qci in range(len(q_starts)):
                nbat = -(-exts[qci] // MEGA)
                for pair in range(2):
                    for bi in range(nbat):
                        stream.append((qci, pair, bi))
            req = {}
            for idx, (qci, pair, bi) in enumerate(stream):
                if ("q", qci) not in req:
                    req[("q", qci)] = idx
                for t in range(bi * MEGA, min(bi * MEGA + MEGA, exts[qci])):
                    req.setdefault(("kv", t // 4), idx)
            # backward-greedy: each subtask takes the latest free slot at or
            # before its deadline, spreading prep into earlier idle windows.
            all_subs = []  # (deadline_slot, order, sub)
            order = 0
            for task, r in sorted(req.items(), key=lambda kv: kv[1]):
                if task[0] == "kv":
                    g = task[1]
                    subs = [("kvt", g, 0), ("kvt", g, 1), ("kvp", g),
                            ("kvv", g)]
                else:
                    qci = task[1]
                    subs = [("qt", qci, 0), ("qt", qci, 1), ("qp", qci)]
                k = len(subs)
                for i, sub in enumerate(subs):
                    all_subs.append((r - 2 - (k - 1 - i), order, sub))
                    order += 1
            emit_at = {}
            pre_tasks = []
            used = set()
            for deadline, order, sub in sorted(all_subs,
                                               key=lambda x: (-x[0], -x[1])):
                s = deadline
                while s >= 0 and s in used:
                    s -= 1
                if s < 0:
                    pre_tasks.append((order, sub))
                else:
                    used.add(s)
                    emit_at.setdefault(s, []).append(sub)
            pre_tasks = [s for _, s in sorted(pre_tasks)]
            for lst in emit_at.values():
                lst.sort(key=lambda sub: next(
                    o for d, o, s2 in all_subs if s2 == sub))
            state = {"gidx": 0}

            def emit_tasks():
                for sub in emit_at.get(state["gidx"], ()):
                    SUBS[sub[0]](*sub[1:])
                state["gidx"] += 1

            for i, sub in enumerate(pre_tasks):
                if i == 2:
                    late_weight_loads()
                SUBS[sub[0]](*sub[1:])
            if len(pre_tasks) <= 2:
                late_weight_loads()

            # epilogue stage-1 (out-proj + residual + bn stats), per chunk
            mvs = pp.tile([128, nqt, 2], F32, tag="mvs")
            xs = [None] * nqt

            def epi_stage2(gt0, gt1):
                n = gt1 - gt0
                lnv = sm.tile([128, n], F32, tag="lnv")
                nc.scalar.activation(lnv, mvs[:, gt0:gt1, 1], AF.Ln,
                                     bias=eps_t, scale=1.0)
                rstd = sm.tile([128, n], F32, tag="rstd")
                nc.scalar.activation(rstd, lnv, AF.Exp, scale=-0.5)
                for gt in range(gt0, gt1):
                    x = xs[gt]
                    nc.vector.tensor_scalar(
                        out=x, in0=x, scalar1=mvs[:, gt, 0:1],
                        scalar2=rstd[:, gt - gt0:gt - gt0 + 1],
                        op0=ALU.subtract, op1=ALU.mult)
                    if need_gb:
                        nc.vector.tensor_mul(x, x, g_b)
                        nc.vector.tensor_add(x, x, b_b)
                    nc.sync.dma_start(out=d_out[gt * 128:(gt + 1) * 128, :],
                                      in_=x)

            def epi_stage1(qci):
                for st in range(SQC // 128):
                    gt = qci * (SQC // 128) + st
                    mh = pr.tile([128, D], F32, tag="t", name=f"mh_{gt}")
                    for c in range(2):
                        nc.tensor.matmul(
                            mh, _r(ctxT[:, c, gt * 128:(gt + 1) * 128]),
                            _r(wo_sb[:, c, :]), start=(c == 0), stop=(c == 1))
                    qnat = qn.tile([128, D], F32, tag="qn")
                    nc.sync.dma_start(out=qnat,
                                      in_=d_q[gt * 128:(gt + 1) * 128, :])
                    x = xres.tile([128, D], F32, tag="x", name=f"x_{gt}")
                    nc.vector.tensor_add(x, mh, qnat)
                    if need_bo:
                        nc.vector.tensor_add(x, x, bo_b)
                    stats = sm.tile([128, 6], F32, tag="st")
                    nc.vector.bn_stats(out=stats, in_=x)
                    nc.vector.bn_aggr(out=mvs[:, gt, :], in_=stats)
                    xs[gt] = x

            # ---- main attention loop ----
            # PV matmuls run one batch behind QK/exp (software pipeline) so
            # the in-order PE stream never stalls on the current batch's exp.
            for qci, q0 in enumerate(q_starts):
                ext = (q0 + SQC) // KVT
                for pair in range(2):
                    ctx_ps = [cpool[par].tile([65, SQC], F32, tag="t",
                                              name=f"ctx_{qci}_{pair}_{par}")
                              for par in range(2)]
                    batches = [list(range(s0, min(s0 + MEGA, ext)))
                               for s0 in range(0, ext, MEGA)]

                    def lo_of(j):
                        # first q-col this kv tile can attend to; narrow the
                        # QK/exp/PV column range when >=256 cols remain
                        lo = max(0, j * KVT - q0)
                        return lo if SQC - lo >= 256 else 0

                    def flush(Ps, batch, first, last):
                        nb = len(batch)
                        for par in range(2):
                            for jj, j in enumerate(batch):
                                lo = lo_of(j)
                                nc.tensor.matmul(
                                    ctx_ps[par][:, lo:],
                                    _r(vsb[:, j, pair * 2 + par, :]),
                                    _r(Ps[par][:, jj, lo:]),
                                    start=(first and jj == 0),
                                    stop=(last and jj == nb - 1))

                    pend = None
                    for bi, batch in enumerate(batches):
                        nb = len(batch)
                        Ps = []
                        los = [lo_of(j) for j in batch]
                        for par in range(2):
                            mega = mpool[par].tile([128, nb, SQC], F32, tag="t")
                            for jj, j in enumerate(batch):
                                lo = los[jj]
                                nc.tensor.matmul(
                                    mega[:, jj, lo:],
                                    _r(kT[64 * par:64 * par + 64, pair,
                                          j * KVT:(j + 1) * KVT]),
                                    _r(qT[64 * par:64 * par + 64, pair,
                                          qci * SQC + lo:(qci + 1) * SQC]),
                                    start=True, stop=True)
                            P = ppool[par].tile([128, MEGA, SQC], F32R, tag="P")
                            if not any(los):
                                nc.scalar.activation(P[:, :nb, :], mega,
                                                     AF.Exp, scale=0.125)
                            else:
                                for jj in range(nb):
                                    lo = los[jj]
                                    nc.scalar.activation(
                                        P[:, jj, lo:], mega[:, jj, lo:],
                                        AF.Exp, scale=0.125)
                            for jj, j in enumerate(batch):
                                if (j + 1) * KVT > q0:  # diagonal tile
                                    lo = los[jj]
                                    nc.gpsimd.affine_select(
                                        out=P[:, jj, lo:], in_=P[:, jj, lo:],
                                        compare_op=ALU.is_ge, fill=0.0,
                                        base=q0 + lo - j * KVT,
                                        channel_multiplier=-1,
                                        pattern=[[1, SQC - lo]])
                            Ps.append(P)
                        if pend is not None:
                            flush(*pend, first=(bi == 1), last=False)
                        pend = (Ps, batch)
                        emit_tasks()
                    flush(*pend, first=(len(batches) == 1), last=True)

                    # normalize by 1/denom and store ctx^T
                    for par in range(2):
                        dsb = smb.tile([1, SQC], F32, tag="dsb")
                        nc.vector.tensor_copy(dsb, ctx_ps[par][64:65, :])
                        rec = smb.tile([1, SQC], F32, tag="rec")
                        nc.vector.reciprocal(rec, dsb)
                        rb = smb.tile([64, SQC], F32, tag="rb")
                        nc.gpsimd.partition_broadcast(rb, rec, channels=64)
                        nc.vector.tensor_mul(
                            ctxT[64 * par:64 * par + 64, pair,
                                 qci * SQC:(qci + 1) * SQC],
                            ctx_ps[par][0:64, :], rb)

                epi_stage1(qci)
                if qci == 2:
                    epi_stage2(0, 3 * (SQC // 128))

            epi_stage2(3 * (SQC // 128), nqt)


# ---------------------------------------------------------------------------
# host side
# ---------------------------------------------------------------------------

_CACHE = {}


def _prep_weights(Wq, bq, Wk, bk, Wv, bv, Wo, bo, gamma, beta):
    wq = np.ascontiguousarray(Wq.reshape(D, 256), np.float32)
    wk = np.ascontiguousarray(Wk.reshape(D, 256), np.float32)
    wv = np.zeros((D, 260), np.float32)
    wv.reshape(D, 4, 65)[:, :, :64] = Wv.reshape(D, 4, 64)
    wo = np.ascontiguousarray(Wo.reshape(256, D).reshape(2, 128, D), np.float32)
    bq2 = np.ascontiguousarray(bq.reshape(2, 128), np.float32)
    bk2 = np.ascontiguousarray(bk.reshape(2, 128), np.float32)
    # bv folds past attention:  attn_norm @ (v + bv) = attn_norm @ v + bv
    # then (ctx + bv) @ Wo + bo = ctx@Wo + (bv@Wo + bo)
    bo_eff = (bv.reshape(256) @ Wo.reshape(256, D) + bo).astype(np.float32)
    need_bo = bool(np.any(bo_eff))
    need_gb = bool(np.any(gamma != 1.0) or np.any(beta != 0.0))
    return dict(wq=wq, wk=wk, wv=wv, wo=wo, bq=bq2, bk=bk2,
                ones4=np.ones((1, 4), np.float32),
                bo_eff=bo_eff.reshape(1, D),
                gamma=np.ascontiguousarray(gamma.reshape(1, D), np.float32),
                beta=np.ascontiguousarray(beta.reshape(1, D), np.float32),
                need_bo=need_bo, need_gb=need_gb)


def _get_programs(need_bo, need_gb):
    key = (need_bo, need_gb)
    if key not in _CACHE:
        ncs = {}
        for half in (0, 1):
            ncs[half] = build_program(Q_STARTS[half], NKV_H[half],
                                      need_bo, need_gb)
        _CACHE[key] = ncs
    return _CACHE[key]


def _make_runner(nc, devices):
    """Build a jit'd shard_map runner for `nc` over a specific device list.

    Mirrors concourse.bass2jax.run_bass_via_pjrt but with an explicit device
    subset so two programs can run concurrently on disjoint cores.
    """
    import jax
    from jax.experimental.shard_map import shard_map
    from jax.sharding import Mesh, PartitionSpec
    from concourse.bass2jax import (_bass_exec_p, install_neuronx_cc_hook,
                                    partition_id_tensor)

    install_neuronx_cc_hook()
    n_cores = len(devices)

    partition_name = (nc.partition_id_tensor.name
                      if nc.partition_id_tensor else None)
    in_names, out_names, out_avals, zero_outs = [], [], [], []
    for alloc in nc.m.functions[0].allocations:
        if not isinstance(alloc, mybir.MemoryLocationSet):
            continue
        name = alloc.memorylocations[0].name
        if alloc.kind == "ExternalInput":
            if name != partition_name:
                in_names.append(name)
        elif alloc.kind == "ExternalOutput":
            out_names.append(name)
            shape = tuple(alloc.tensor_shape)
            dtype = mybir.dt.np(alloc.dtype)
            out_avals.append(jax.core.ShapedArray(shape, dtype))
            zero_outs.append(np.zeros(shape, dtype))
    n_params = len(in_names)
    all_names = in_names + out_names
    if partition_name is not None:
        all_names = all_names + [partition_name]

    def _body(*args):
        operands = list(args)
        if partition_name is not None:
            operands.append(partition_id_tensor())
        outs = _bass_exec_p.bind(
            *operands, out_avals=tuple(out_avals), in_names=tuple(all_names),
            out_names=tuple(out_names), lowering_input_output_aliases=(),
            sim_require_finite=True, sim_require_nnan=True, nc=nc)
        return tuple(outs)

    donate = tuple(range(n_params, n_params + len(out_names)))
    mesh = Mesh(np.asarray(devices), ("core",))
    in_specs = (PartitionSpec("core"),) * (n_params + len(out_names))
    out_specs = (PartitionSpec("core"),) * len(out_names)
    fn = jax.jit(shard_map(_body, mesh=mesh, in_specs=in_specs,
                           out_specs=out_specs, check_rep=False),
                 donate_argnums=donate, keep_unused=True)

    def run(in_maps):
        assert len(in_maps) == n_cores
        concat_in = [np.concatenate([np.asarray(m[n]) for m in in_maps], axis=0)
                     for n in in_names]
        concat_zero = [np.zeros((n_cores * z.shape[0], *z.shape[1:]), z.dtype)
                       for z in zero_outs]
        out_arrs = fn(*concat_in, *concat_zero)
        return out_arrs, out_names, out_avals

    run.fn = fn
    run.mesh = mesh
    run.in_names = in_names
    run.zero_outs = zero_outs
    run.n_cores = n_cores
    return run


_RUNNERS = {}
_LAST_IN_MAPS = {}


def kernel(query, value, Wq, bq, Wk, bk, Wv, bv, Wo, bo, gamma, beta):
    import jax
    query = np.asarray(query, np.float32)
    value = np.asarray(value, np.float32)
    w = _prep_weights(np.asarray(Wq), np.asarray(bq), np.asarray(Wk),
                      np.asarray(bk), np.asarray(Wv), np.asarray(bv),
                      np.asarray(Wo), np.asarray(bo), np.asarray(gamma),
                      np.asarray(beta))
    ncs = _get_programs(w["need_bo"], w["need_gb"])

    rkey = (w["need_bo"], w["need_gb"])
    if rkey not in _RUNNERS:
        devs = jax.devices()
        _RUNNERS[rkey] = {
            0: _make_runner(ncs[0], devs[0:4]),
            1: _make_runner(ncs[1], devs[4:8]),
        }
    runners = _RUNNERS[rkey]

    wmap = {k: w[k] for k in ("wq", "wk", "wv", "wo", "bq", "bk", "bo_eff",
                              "gamma", "beta", "ones4")}
    in_maps = {0: [], 1: []}
    for half in (0, 1):
        for b in range(B):
            qs = np.concatenate([query[b, q0:q0 + SQC]
                                 for q0 in Q_STARTS[half]], axis=0)
            m = dict(wmap)
            m["q_shard"] = np.ascontiguousarray(qs)
            m["value"] = np.ascontiguousarray(value[b, :NKV_H[half] * KVT])
            in_maps[half].append(m)
        _LAST_IN_MAPS[half] = in_maps[half]

    results = {}

    def _dispatch(half):
        results[half] = runners[half].__call__(in_maps[half])

    # dispatch A then B; jax dispatch is async so both halves overlap on
    # their disjoint device sets. Use threads to overlap even blocking parts.
    th = threading.Thread(target=_dispatch, args=(0,))
    th.start()
    _dispatch(1)
    th.join()

    out = np.empty((B, S, D), np.float32)
    for half in (0, 1):
        arrs, out_names, out_avals = results[half]
        oi = out_names.index("out")
        full = np.asarray(arrs[oi]).reshape(4, len(Q_STARTS[half]) * SQC, D)
        for b in range(B):
            for ci, q0 in enumerate(Q_STARTS[half]):
                out[b, q0:q0 + SQC] = full[b, ci * SQC:(ci + 1) * SQC]
    return out

